# revision 1
# baseline (speedup 1.0000x reference)
"""Distributed self-attention kernel for Trainium2, 8 NeuronCores.

Sequence-parallel (the module's own sharding): S=3072 is sharded 384
rows/core. Each core computes its q/k/v projection chunk in bf16 (fp32
PSUM accumulation); two AllGathers (v first, then k^T — each ~0.77
MB/rank in bf16) share K/V, then each core runs attention for its 384
queries over all 3072 keys and writes its slice of the output.

Attention runs in transposed-score layout (s^T[key, query]) so P@V needs
no transpose of P. All matmul stationaries are full 128x128 (partial
stationaries stream at half rate on TRN2):
  - scores: stationary = k^T pair-block [128 dims, 128 keys]; the query
    rhs is zero-padded per head ([q_even; 0] / [0; q_odd]) so each head's
    scores come out exact at full rate.
  - P@V: stationary = the contiguous 128-col window [v_h | ones | ...]
    of a 65-interleaved v tile; output rows 0-63 are context, row 64
    accumulates the softmax denominator, rows 65-127 catch garbage from
    the next head's v and are never read.
exp() on ScalarE with the 1/sqrt(64) scale fused (no max subtraction:
softmax logits here are |qk/8| < ~4, exp is safely in fp32/bf16 range —
mathematically identical to the reference's max-subtracted softmax).
Even/odd heads are staggered so ScalarE exp and PE matmuls overlap within
exactly 8 PSUM banks. PE-transpose + VectorE normalize finish each head;
the core DMAs out its [384, 1024] slice and the host concatenates.
"""

import numpy as np
import ml_dtypes

import concourse.bacc as bacc
import concourse.mybir as mybir
import concourse.tile as tile
from concourse import bass_utils

F32 = mybir.dt.float32
BF16 = mybir.dt.bfloat16
AF = mybir.ActivationFunctionType

N_CORES = 8
B, S, HID = 1, 3072, 1024
NH, HD = 16, 64
SC = S // N_CORES          # 384 sequence rows per core
QT = SC // 128             # 3 query sub-tiles per core
KT = S // 128              # 24 key tiles globally
KIN = HID + 1              # augmented contraction (bias row)
NG = KT // 3               # 8 groups of 3 key tiles for batched exp
VW = NH * 65               # interleaved v width: [v_h (64) | 1.0] x 16 = 1040
VTW = VW + 63              # v tile width incl. zero tail so head 15's
                           # 128-col stationary window stays in bounds

_KSZ = 8 * 128 * SC        # k^T elements per rank in the AG payload
_VSZ = QT * 128 * VW       # v elements per rank (65-interleaved layout)
_PAYLOAD = _KSZ + _VSZ

_cache: dict = {}


def _build(with_mask: bool):
    nc = bacc.Bacc("TRN2", target_bir_lowering=False, debug=False,
                   num_devices=N_CORES)

    xt = nc.dram_tensor("xt", [KIN, SC], BF16, kind="ExternalInput")
    w = nc.dram_tensor("w", [3, KIN, HID], BF16, kind="ExternalInput")
    ident = nc.dram_tensor("ident", [128, 128], F32, kind="ExternalInput")
    if with_mask:
        maskt = nc.dram_tensor("maskt", [128, KT], F32, kind="ExternalInput")
    out = nc.dram_tensor("out", [SC, HID], F32, kind="ExternalOutput")

    with tile.TileContext(nc) as tc:
        with (
            tc.tile_pool(name="persist", bufs=1) as pp,
            tc.tile_pool(name="dram", bufs=1, space="DRAM") as dram,
        ):
            kin = dram.tile([_KSZ], BF16)
            kout = dram.tile([N_CORES, _KSZ], BF16, addr_space="Shared")
            vin = dram.tile([_VSZ], BF16)
            vout = dram.tile([N_CORES, _VSZ], BF16, addr_space="Shared")

            # ---- persistent SBUF tensors ----
            xsb = pp.tile([128, 9 * SC], BF16)       # x^T, 9 contraction slices
            qz = [pp.tile([128, 2 * SC], BF16, name=f"qz{m}") for m in range(8)]
            ksb = [pp.tile([128, S], BF16, name=f"ksb{h}") for h in range(8)]
            vsb = [pp.tile([128, VTW], BF16, name=f"vsb{k}") for k in range(KT)]
            idsb = pp.tile([128, 128], F32)
            ctxsb = [pp.tile([65, SC], F32, name=f"ctxsb{h}") for h in range(NH)]
            osb = [pp.tile([128, HID], F32, name=f"osb{t}") for t in range(QT)]
            if with_mask:
                msb = pp.tile([128, KT], F32)
                nc.sync.dma_start(msb[:], maskt[:])

            # x^T load: 8 full slices + 1-row bias slice
            for j in range(8):
                nc.sync.dma_start(xsb[:, j * SC:(j + 1) * SC],
                                  xt[j * 128:(j + 1) * 128, :])
            nc.sync.dma_start(xsb[0:1, 8 * SC:9 * SC], xt[1024:1025, :])

            # zero-pad q pair tiles (must precede the q-projection copies)
            for m in range(8):
                nc.vector.memset(qz[m][:], 0.0)

            # ---- phase A: projections ----
            with (
                tc.tile_pool(name="wpool", bufs=6) as wpool,
                tc.tile_pool(name="stg", bufs=4) as stg,
                tc.tile_pool(name="ppsum", bufs=4, space="PSUM") as ppsum,
            ):
                def load_w(proj, j):
                    if j < 8:
                        wt = wpool.tile([128, HID], BF16, tag="w", bufs=12,
                                        name=f"w{proj}_{j}")
                        nc.sync.dma_start(wt[:], w[proj, j * 128:(j + 1) * 128, :])
                    else:
                        wt = wpool.tile([1, HID], BF16, tag="wb", bufs=3,
                                        name=f"wb{proj}")
                        nc.sync.dma_start(wt[:], w[proj, HID:HID + 1, :])
                    return wt

                # v first: every pair's PV walks all of V from its
                # first group, while k^T is consumed progressively per pair
                # — so the k AllGather can finish second without stalling.
                wv = [load_w(2, j) for j in range(9)]
                for st in range(QT):
                    # staging tile already 65-interleaved with the ones
                    # columns, so post-AG v loads are fully contiguous
                    vst = stg.tile([128, VW], BF16, tag="vst", name=f"vst{st}")
                    vst3 = vst.rearrange("p (h y) -> p h y", y=65)
                    nc.vector.memset(vst3[:, :, 64:65], 1.0)
                    for half in range(2):
                        pv = ppsum.tile([128, 512], F32, tag="pv", bufs=3,
                                        name=f"pv{st}_{half}")
                        for j in range(9):
                            rows = 128 if j < 8 else 1
                            nc.tensor.matmul(
                                pv[:],
                                xsb[:rows, j * SC + st * 128: j * SC + (st + 1) * 128],
                                wv[j][:rows, half * 512:(half + 1) * 512],
                                start=(j == 0), stop=(j == 8))
                        nc.vector.tensor_copy(
                            vst3[:, half * 8:(half + 1) * 8, 0:64],
                            pv[:].rearrange("p (h y) -> p h y", y=HD))
                    nc.sync.dma_start(
                        vin[st * 128 * VW:(st + 1) * 128 * VW]
                        .rearrange("(p x) -> p x", x=VW), vst[:])

                nc.gpsimd.collective_compute(
                    "AllGather",
                    mybir.AluOpType.bypass,
                    replica_groups=[list(range(N_CORES))],
                    ins=[vin[:].opt()],
                    outs=[vout[:].opt()],
                )

                wk = [load_w(1, j) for j in range(9)]
                for m in range(8):
                    pk = ppsum.tile([128, SC], F32, tag="pk", bufs=5, name=f"pk{m}")
                    for j in range(9):
                        rows = 128 if j < 8 else 1
                        nc.tensor.matmul(
                            pk[:], wk[j][:rows, m * 128:(m + 1) * 128],
                            xsb[:rows, j * SC:(j + 1) * SC],
                            start=(j == 0), stop=(j == 8))
                    kst = stg.tile([128, SC], BF16, tag="kst", name=f"kst{m}")
                    nc.vector.tensor_copy(kst[:], pk[:])
                    nc.sync.dma_start(
                        kin[m * 128 * SC:(m + 1) * 128 * SC]
                        .rearrange("(p x) -> p x", x=SC), kst[:])

                nc.gpsimd.collective_compute(
                    "AllGather",
                    mybir.AluOpType.bypass,
                    replica_groups=[list(range(N_CORES))],
                    ins=[kin[:].opt()],
                    outs=[kout[:].opt()],
                )

                wq = [load_w(0, j) for j in range(9)]
                for m in range(8):
                    pq = ppsum.tile([128, SC], F32, tag="pk", bufs=5, name=f"pq{m}")
                    for j in range(9):
                        rows = 128 if j < 8 else 1
                        nc.tensor.matmul(
                            pq[:], wq[j][:rows, m * 128:(m + 1) * 128],
                            xsb[:rows, j * SC:(j + 1) * SC],
                            start=(j == 0), stop=(j == 8))
                    # zero-padded halves: head-even in cols 0:SC (rows 0-63),
                    # head-odd in cols SC:2SC (rows 64-127)
                    nc.vector.tensor_copy(qz[m][0:64, 0:SC], pq[0:64, :])
                    nc.vector.tensor_copy(qz[m][64:128, SC:2 * SC], pq[64:128, :])

            # late prologue pieces, needed only by phase C/D — emitted
            # after the projection/AllGather chain so they don't compete
            nc.sync.dma_start(idsb[:], ident[:])
            for k in range(KT):
                nc.vector.memset(vsb[k][:, VW:VTW], 0.0)

            # ---- phase B: spread gathered K/V into SBUF ----
            # ALL v loads are emitted before any k load: the SP sequencer is
            # a FIFO, and the v AllGather finishes ~35us before the k one —
            # v triggers parked behind a k-wait would burn that head start.
            # k triggers can't fire before their AllGather lands anyway.
            for k in range(KT):
                r, st = k // QT, k % QT
                nc.sync.dma_start(
                    vsb[k][:, 0:VW],
                    vout[r, st * 128 * VW:(st + 1) * 128 * VW]
                    .rearrange("(p x) -> p x", x=VW))
            for hp in range(8):
                for r in range(N_CORES):
                    nc.sync.dma_start(
                        ksb[hp][:, r * SC:(r + 1) * SC],
                        kout[r, hp * 128 * SC:(hp + 1) * 128 * SC]
                        .rearrange("(p x) -> p x", x=SC))

            # ---- phase C: attention, staggered even/odd heads ----
            with (
                tc.tile_pool(name="spoolE", bufs=1, space="PSUM") as spoolE,
                tc.tile_pool(name="spoolO", bufs=1, space="PSUM") as spoolO,
                tc.tile_pool(name="cpool", bufs=1, space="PSUM") as cpool,
                tc.tile_pool(name="ppool", bufs=10) as ppool,
            ):
                def score_block(sp, hp, e, g):
                    for j in range(3):
                        kt = g * 3 + j
                        nc.tensor.matmul(
                            sp[:, j * 512: j * 512 + SC],
                            ksb[hp][:, kt * 128:(kt + 1) * 128],
                            qz[hp][:, e * SC:(e + 1) * SC],
                            start=True, stop=True)

                def exp_block(pt, sp, g):
                    src3 = sp.rearrange("p (g x) -> p g x", x=512)[:, :, 0:SC]
                    dst3 = pt.rearrange("p (g x) -> p g x", x=SC)
                    if with_mask:
                        for j in range(3):
                            kt = g * 3 + j
                            nc.scalar.activation(
                                dst3[:, j, :], src3[:, j, :], AF.Exp,
                                bias=msb[:, kt:kt + 1], scale=0.125)
                    else:
                        nc.scalar.activation(dst3, src3, AF.Exp, scale=0.125)

                def pv_block(ctx, pt, h, g):
                    # stationary = contiguous [v_h | ones | v_{h+1}...] window;
                    # out rows 0-63 = ctx, row 64 = denominator, rows 65-127
                    # accumulate next-head garbage that is never read.
                    for j in range(3):
                        kt = g * 3 + j
                        nc.tensor.matmul(
                            ctx[:], vsb[kt][:, 65 * h: 65 * h + 128],
                            pt[:, j * SC:(j + 1) * SC],
                            start=(g == 0 and j == 0),
                            stop=(g == NG - 1 and j == 2))

                for hp in range(8):
                    ctxE = cpool.tile([128, SC], F32, tag="ctxE", name=f"cE{hp}")
                    ctxO = cpool.tile([128, SC], F32, tag="ctxO", name=f"cO{hp}")
                    for g in range(NG):
                        spE = spoolE.tile([128, 1536], F32, tag="spE",
                                          name=f"spE{hp}_{g}")
                        score_block(spE, hp, 0, g)
                        ptE = ppool.tile([128, 3 * SC], BF16, tag="pt",
                                         name=f"ptE{hp}_{g}")
                        exp_block(ptE, spE, g)
                        spO = spoolO.tile([128, 1536], F32, tag="spO",
                                          name=f"spO{hp}_{g}")
                        score_block(spO, hp, 1, g)
                        ptO = ppool.tile([128, 3 * SC], BF16, tag="pt",
                                         name=f"ptO{hp}_{g}")
                        exp_block(ptO, spO, g)
                        pv_block(ctxE, ptE, 2 * hp, g)
                        pv_block(ctxO, ptO, 2 * hp + 1, g)
                    nc.vector.tensor_copy(ctxsb[2 * hp][:], ctxE[0:65, :])
                    nc.vector.tensor_copy(ctxsb[2 * hp + 1][:], ctxO[0:65, :])

            # ---- phase D: transpose back, normalize, store ----
            with (
                tc.tile_pool(name="tpool", bufs=8, space="PSUM") as tpool,
                tc.tile_pool(name="rpool2", bufs=8) as rpool2,
            ):
                for h in range(NH):
                    for t in range(QT):
                        tp = tpool.tile([128, 65], F32, tag="tp",
                                        name=f"tp{h}_{t}")
                        nc.tensor.transpose(
                            tp[:], ctxsb[h][:, t * 128:(t + 1) * 128],
                            idsb[0:65, 0:65])
                        rec = rpool2.tile([128, 1], F32, tag="rec",
                                          name=f"rec{h}_{t}")
                        nc.vector.reciprocal(rec[:], tp[:, 64:65])
                        nc.vector.tensor_scalar_mul(
                            osb[t][:, h * HD:(h + 1) * HD], tp[:, 0:64], rec[:])
                for t in range(QT):
                    nc.sync.dma_start(out[t * 128:(t + 1) * 128, :], osb[t][:])

    nc.compile()
    return nc


def _get_program(with_mask: bool):
    key = ("prog", with_mask)
    if key not in _cache:
        _cache[key] = _build(with_mask)
    return _cache[key]


def kernel(hidden_states, attention_mask, Wq, bq, Wk, bk, Wv, bv):
    x = np.asarray(hidden_states, np.float32).reshape(S, HID)
    mask = np.asarray(attention_mask, np.float32).reshape(-1)
    if mask.size == 1:
        mask = np.full(S, float(mask[0]), np.float32)
    with_mask = bool(np.any(mask))

    # augmented weights: [3, 1025, 1024] with the bias as the last
    # contraction row; x^T gets a matching ones row.
    w_aug = np.empty((3, KIN, HID), np.float32)
    for i, (W, b) in enumerate(((Wq, bq), (Wk, bk), (Wv, bv))):
        w_aug[i, :HID] = np.asarray(W, np.float32).T
        w_aug[i, HID] = np.asarray(b, np.float32)
    w_aug = w_aug.astype(ml_dtypes.bfloat16)

    ident = np.eye(128, dtype=np.float32)

    nc = _get_program(with_mask)
    in_maps = []
    for c in range(N_CORES):
        xtc = np.empty((KIN, SC), np.float32)
        xtc[:HID] = x[c * SC:(c + 1) * SC, :].T
        xtc[HID] = 1.0
        m = {
            "xt": xtc.astype(ml_dtypes.bfloat16),
            "w": w_aug,
            "ident": ident,
        }
        if with_mask:
            m["maskt"] = np.ascontiguousarray(
                mask.reshape(KT, 128).T.astype(np.float32))
        in_maps.append(m)

    _cache["last_in_maps"] = in_maps
    res = bass_utils.run_bass_kernel_spmd(nc, in_maps, core_ids=list(range(N_CORES)))
    out = np.concatenate([res.results[c]["out"] for c in range(N_CORES)], axis=0)
    return out.reshape(B, S, HID).astype(np.float32)



# revision 2
# speedup vs baseline: 1.3041x; 1.3041x over previous
"""Distributed self-attention kernel for Trainium2, 8 NeuronCores.

Head-parallel sharding (v2): NH=16 heads across 8 cores = one even/odd
head pair per core. Each core computes q/k/v projections for ITS pair
over the FULL sequence from the full hidden states (replicated; the
6.3 MB x^T load streams in 512-column blocks and overlaps the
projection matmuls), runs attention for its 2 heads over all 3072
queries x 3072 keys, and writes its [3072, 128] slice of the hidden
dim. No collectives at all -- the v1 sequence-parallel design spent
~80us serialized in two AllGathers.

Attention runs in transposed-score layout (s^T[key, query]) so P@V
needs no transpose of P. All matmul stationaries are full 128x128:
  - scores: stationary = k^T pair-block [128 dims, 128 keys]; the query
    rhs is zero-padded per head ([q_even; 0] / [0; q_odd]) so each
    head's scores come out exact at full rate.
  - P@V: stationary = the contiguous 128-col window of the per-key-tile
    [v_even | 1 | v_odd | 1 | 0pad] interleave; output rows 0-63 are
    context, row 64 the softmax denominator, rows 65-127 garbage that
    is never read.
exp() on ScalarE with the 1/sqrt(64) scale fused (no max subtraction:
logits here are |qk/8| < ~4, exp is safely in range -- mathematically
identical to the reference's max-subtracted softmax). Queries are
processed in 8 blocks of 384; even/odd heads are staggered inside each
block so ScalarE exp and PE matmuls overlap within exactly 8 PSUM
banks. PE-transpose + VectorE normalize finish each head; the core
DMAs out its [3072, 128] slice and the host concatenates on hidden.
"""

import numpy as np
import ml_dtypes

import concourse.bacc as bacc
import concourse.mybir as mybir
import concourse.tile as tile
from concourse import bass_utils

F32 = mybir.dt.float32
BF16 = mybir.dt.bfloat16
AF = mybir.ActivationFunctionType

N_CORES = 8
B, S, HID = 1, 3072, 1024
NH, HD = 16, 64
KIN = HID + 1               # augmented contraction (bias row)
KT = S // 128               # 24 key tiles
CB = 6                      # x streamed in 6 blocks of 512 columns
QB = 8                      # 8 query blocks of 384
QW = S // QB                # 384 queries per block
NG = KT // 3                # 8 groups of 3 key tiles for batched exp
VTW = 208                   # per-kt v stride: [v_e(64) | 1 | v_o(64) | 1 | pad]

_cache: dict = {}


def _build(with_mask: bool):
    nc = bacc.Bacc("TRN2", target_bir_lowering=False, debug=False,
                   num_devices=N_CORES)

    xt = nc.dram_tensor("xt", [KIN, S], BF16, kind="ExternalInput")
    w = nc.dram_tensor("w", [3, KIN, 128], BF16, kind="ExternalInput")
    ident = nc.dram_tensor("ident", [128, 128], F32, kind="ExternalInput")
    if with_mask:
        maskt = nc.dram_tensor("maskt", [128, KT], F32, kind="ExternalInput")
    out = nc.dram_tensor("out", [S, 128], F32, kind="ExternalOutput")

    with tile.TileContext(nc) as tc:
        with tc.tile_pool(name="persist", bufs=1) as pp:
            # ---- persistent SBUF tensors ----
            xsb = pp.tile([128, 8 * S], BF16)      # x^T, 8 full slices
            xb = pp.tile([1, S], BF16)             # ones row (bias)
            qz = pp.tile([128, 2 * S], BF16)       # zero-padded q halves
            ksb = pp.tile([128, S], BF16)          # k^T for the pair
            vsb = pp.tile([128, KT * VTW], BF16)   # interleaved v tiles
            idsb = pp.tile([128, 128], F32)
            ctxsb = [pp.tile([65, S], F32, name=f"ctxsb{h}") for h in range(2)]
            osb = pp.tile([128, KT * 128], F32)    # output staging [q, 128]
            if with_mask:
                msb = pp.tile([128, KT], F32)
                nc.sync.dma_start(msb[:], maskt[:])

            # zero halves of qz (Pool engine; off the DVE/Scalar path)
            nc.gpsimd.memset(qz[64:128, 0:S], 0.0)
            nc.gpsimd.memset(qz[0:64, S:2 * S], 0.0)
            # v interleave: ones columns and zero tail
            vsb3 = vsb.rearrange("p (k y) -> p k y", y=VTW)
            nc.gpsimd.memset(vsb3[:, :, 64:65], 1.0)
            nc.gpsimd.memset(vsb3[:, :, 129:130], 1.0)
            nc.gpsimd.memset(vsb3[:, :, 130:VTW], 0.0)
            nc.sync.dma_start(idsb[:], ident[:])

            # ---- phase A: stream x in column blocks, project q/k/v ----
            with (
                tc.tile_pool(name="wpool", bufs=1) as wpool,
                tc.tile_pool(name="ppsum", bufs=2, space="PSUM") as ppsum,
            ):
                wt = [[None] * 9 for _ in range(3)]
                for proj in range(3):
                    for j in range(9):
                        rows = 128 if j < 8 else 1
                        wt[proj][j] = wpool.tile([rows, 128], BF16,
                                                 name=f"w{proj}_{j}")
                        nc.sync.dma_start(
                            wt[proj][j][:],
                            w[proj, j * 128:j * 128 + rows, :])

                for cb in range(CB):
                    c0 = cb * 512
                    for j in range(8):
                        nc.sync.dma_start(xsb[:, j * S + c0:j * S + c0 + 512],
                                          xt[j * 128:(j + 1) * 128, c0:c0 + 512])
                    nc.sync.dma_start(xb[0:1, c0:c0 + 512],
                                      xt[HID:KIN, c0:c0 + 512])

                def xs(j, c0, width):
                    if j < 8:
                        return xsb[:, j * S + c0:j * S + c0 + width]
                    return xb[0:1, c0:c0 + width]

                for cb in range(CB):
                    c0 = cb * 512
                    # q^T and k^T chunks: stationary = w_j, rhs = x^T chunk
                    pq = ppsum.tile([128, 512], F32, tag="pq", name=f"pq{cb}")
                    for j in range(9):
                        nc.tensor.matmul(pq[:], wt[0][j][:], xs(j, c0, 512),
                                         start=(j == 0), stop=(j == 8))
                    nc.vector.tensor_copy(qz[0:64, c0:c0 + 512], pq[0:64, :])
                    nc.vector.tensor_copy(qz[64:128, S + c0:S + c0 + 512],
                                          pq[64:128, :])
                    pk = ppsum.tile([128, 512], F32, tag="pk", name=f"pk{cb}")
                    for j in range(9):
                        nc.tensor.matmul(pk[:], wt[1][j][:], xs(j, c0, 512),
                                         start=(j == 0), stop=(j == 8))
                    nc.scalar.copy(ksb[:, c0:c0 + 512], pk[:])
                    # v chunks: stationary = x^T key window, rhs = w_j
                    pv = ppsum.tile([128, 512], F32, tag="pv", name=f"pv{cb}")
                    for kk in range(4):
                        for j in range(9):
                            nc.tensor.matmul(
                                pv[:, kk * 128:(kk + 1) * 128],
                                xs(j, c0 + kk * 128, 128), wt[2][j][:],
                                start=(j == 0), stop=(j == 8))
                    pv3 = pv.rearrange("p (k y) -> p k y", y=128)
                    vd = vsb3[:, cb * 4:(cb + 1) * 4, :]
                    nc.scalar.copy(vd[:, :, 0:64], pv3[:, :, 0:64])
                    nc.scalar.copy(vd[:, :, 65:129], pv3[:, :, 64:128])

            # ---- phase C: attention, staggered even/odd heads ----
            with (
                tc.tile_pool(name="spoolE", bufs=1, space="PSUM") as spoolE,
                tc.tile_pool(name="spoolO", bufs=1, space="PSUM") as spoolO,
                tc.tile_pool(name="cpool", bufs=1, space="PSUM") as cpool,
                tc.tile_pool(name="ppool", bufs=10) as ppool,
            ):
                def score_block(sp, e, g, q0):
                    for j in range(3):
                        kt = g * 3 + j
                        nc.tensor.matmul(
                            sp[:, j * 512:j * 512 + QW],
                            ksb[:, kt * 128:(kt + 1) * 128],
                            qz[:, e * S + q0:e * S + q0 + QW],
                            start=True, stop=True)

                def exp_block(pt, sp, g):
                    src3 = sp.rearrange("p (g x) -> p g x", x=512)[:, :, 0:QW]
                    dst3 = pt.rearrange("p (g x) -> p g x", x=QW)
                    if with_mask:
                        for j in range(3):
                            kt = g * 3 + j
                            nc.scalar.activation(
                                dst3[:, j, :], src3[:, j, :], AF.Exp,
                                bias=msb[:, kt:kt + 1], scale=0.125)
                    else:
                        nc.scalar.activation(dst3, src3, AF.Exp, scale=0.125)

                def pv_block(ctx, pt, h, g):
                    for j in range(3):
                        kt = g * 3 + j
                        nc.tensor.matmul(
                            ctx[:], vsb[:, kt * VTW + 65 * h:
                                        kt * VTW + 65 * h + 128],
                            pt[:, j * QW:(j + 1) * QW],
                            start=(g == 0 and j == 0),
                            stop=(g == NG - 1 and j == 2))

                for qb in range(QB):
                    q0 = qb * QW
                    ctxE = cpool.tile([128, QW], F32, tag="ctxE", name=f"cE{qb}")
                    ctxO = cpool.tile([128, QW], F32, tag="ctxO", name=f"cO{qb}")
                    for g in range(NG):
                        spE = spoolE.tile([128, 1536], F32, tag="spE",
                                          name=f"spE{qb}_{g}")
                        score_block(spE, 0, g, q0)
                        ptE = ppool.tile([128, 3 * QW], BF16, tag="pt",
                                         name=f"ptE{qb}_{g}")
                        exp_block(ptE, spE, g)
                        spO = spoolO.tile([128, 1536], F32, tag="spO",
                                          name=f"spO{qb}_{g}")
                        score_block(spO, 1, g, q0)
                        ptO = ppool.tile([128, 3 * QW], BF16, tag="pt",
                                         name=f"ptO{qb}_{g}")
                        exp_block(ptO, spO, g)
                        pv_block(ctxE, ptE, 0, g)
                        pv_block(ctxO, ptO, 1, g)
                    nc.vector.tensor_copy(ctxsb[0][:, q0:q0 + QW], ctxE[0:65, :])
                    nc.vector.tensor_copy(ctxsb[1][:, q0:q0 + QW], ctxO[0:65, :])

            # ---- phase D: transpose back, normalize, store ----
            with (
                tc.tile_pool(name="tpool", bufs=8, space="PSUM") as tpool,
                tc.tile_pool(name="rpool2", bufs=8) as rpool2,
            ):
                for t in range(KT):
                    for h in range(2):
                        tp = tpool.tile([128, 65], F32, tag="tp",
                                        name=f"tp{h}_{t}")
                        nc.tensor.transpose(
                            tp[:], ctxsb[h][:, t * 128:(t + 1) * 128],
                            idsb[0:65, 0:65])
                        rec = rpool2.tile([128, 1], F32, tag="rec",
                                          name=f"rec{h}_{t}")
                        nc.vector.reciprocal(rec[:], tp[:, 64:65])
                        nc.vector.tensor_scalar_mul(
                            osb[:, t * 128 + h * 64:t * 128 + h * 64 + 64],
                            tp[:, 0:64], rec[:])
                    nc.sync.dma_start(
                        out[t * 128:(t + 1) * 128, :],
                        osb[:, t * 128:(t + 1) * 128])

    nc.compile()
    return nc


def _get_program(with_mask: bool):
    key = ("prog", with_mask)
    if key not in _cache:
        _cache[key] = _build(with_mask)
    return _cache[key]


def kernel(hidden_states, attention_mask, Wq, bq, Wk, bk, Wv, bv):
    x = np.asarray(hidden_states, np.float32).reshape(S, HID)
    mask = np.asarray(attention_mask, np.float32).reshape(-1)
    if mask.size == 1:
        mask = np.full(S, float(mask[0]), np.float32)
    with_mask = bool(np.any(mask))

    # augmented weights: [3, 1025, 1024] with the bias as the last
    # contraction row; x^T gets a matching ones row.
    w_aug = np.empty((3, KIN, HID), np.float32)
    for i, (W, b) in enumerate(((Wq, bq), (Wk, bk), (Wv, bv))):
        w_aug[i, :HID] = np.asarray(W, np.float32).T
        w_aug[i, HID] = np.asarray(b, np.float32)
    w_aug = w_aug.astype(ml_dtypes.bfloat16)

    xtc = np.empty((KIN, S), np.float32)
    xtc[:HID] = x.T
    xtc[HID] = 1.0
    xtc = xtc.astype(ml_dtypes.bfloat16)
    ident = np.eye(128, dtype=np.float32)
    if with_mask:
        maskt = np.ascontiguousarray(
            mask.reshape(KT, 128).T.astype(np.float32))

    nc = _get_program(with_mask)
    in_maps = []
    for c in range(N_CORES):
        m = {
            "xt": xtc,
            "w": np.ascontiguousarray(w_aug[:, :, c * 128:(c + 1) * 128]),
            "ident": ident,
        }
        if with_mask:
            m["maskt"] = maskt
        in_maps.append(m)

    _cache["last_in_maps"] = in_maps
    res = bass_utils.run_bass_kernel_spmd(nc, in_maps, core_ids=list(range(N_CORES)))
    out = np.concatenate([res.results[c]["out"] for c in range(N_CORES)], axis=1)
    return out.reshape(B, S, HID).astype(np.float32)


# revision 4
# speedup vs baseline: 1.6207x; 1.2428x over previous
"""Distributed self-attention kernel for Trainium2, 8 NeuronCores.

Head-parallel sharding: NH=16 heads across 8 cores = one even/odd head
pair per core. Each core computes q/k/v projections for ITS pair over
the FULL sequence from the full hidden states (replicated; the 6.3 MB
x^T load streams in 512-column blocks and overlaps the projection
matmuls), runs attention for its 2 heads over all 3072 queries x 3072
keys, and writes its [3072, 128] slice of the hidden dim. No
collectives at all.

Pipeline notes:
  - x^T DMA triggers issue from the Pool sequencer (cheap dispatch),
    emitted before everything else.
  - No bias matmuls: bq/bk are folded into the PSUM->SBUF copies; bv is
    added on the host (ctx/denom + bv is exact since sum_k p_k = 1).
  - Scores in transposed layout (s^T[key, query]): stationary = k^T
    pair-block [128 dims, 128 keys], query rhs zero-padded per head
    ([q_even; 0] / [0; q_odd]) so each head streams at full PE rate.
  - exp on ScalarE with the 1/sqrt(64) scale fused (no max subtraction:
    logits are small; mathematically identical to the reference).
  - P@V uses exp'd score tiles as the STATIONARY operand and v columns
    as the moving operand: out accumulates directly in [query, 65]
    layout (64 ctx dims + the softmax denominator from the interleaved
    ones column), so no PE transposes and no PSUM->SBUF ctx copies are
    needed. All six [128q, 65] accumulators of a query block live in
    ONE PSUM bank: only the first matmul of the block carries
    start=True (the hardware clears has_written bank-wide), every later
    matmul accumulates-or-overwrites per element; only the last carries
    stop=True. ctx banks double-buffer across query blocks so the
    VectorE normalize epilogue of block qb overlaps block qb+1.
"""

import numpy as np
import ml_dtypes

import concourse.bacc as bacc
import concourse.mybir as mybir
import concourse.tile as tile
from concourse import bass_utils

F32 = mybir.dt.float32
BF16 = mybir.dt.bfloat16
AF = mybir.ActivationFunctionType

N_CORES = 8
B, S, HID = 1, 3072, 1024
NH, HD = 16, 64
KT = S // 128               # 24 key tiles
CB = 6                      # x streamed in 6 blocks of 512 columns
QB = 8                      # 8 query blocks of 384
QW = S // QB                # 384 queries per block
NG = KT // 3                # 8 groups of 3 key tiles for batched exp
VTW = 208                   # per-kt v stride: [v_e(64) | 1 | v_o(64) | 1 | pad]

_cache: dict = {}


def _build(with_mask: bool):
    nc = bacc.Bacc("TRN2", target_bir_lowering=False, debug=False,
                   num_devices=N_CORES)

    xt = nc.dram_tensor("xt", [HID, S], BF16, kind="ExternalInput")
    w = nc.dram_tensor("w", [3, HID, 128], BF16, kind="ExternalInput")
    bcol = nc.dram_tensor("bcol", [128, 2], F32, kind="ExternalInput")
    if with_mask:
        maskt = nc.dram_tensor("maskt", [128, KT], F32, kind="ExternalInput")
    out = nc.dram_tensor("out", [S, 128], F32, kind="ExternalOutput")

    with tile.TileContext(nc) as tc:
        with tc.tile_pool(name="persist", bufs=1) as pp:
            # ---- persistent SBUF tensors ----
            xsb = pp.tile([128, 8 * S], BF16, tag="xsb")
            qz = pp.tile([128, 2 * S], BF16, tag="qz")
            ksb = pp.tile([128, S], BF16, tag="ksb")
            vsb = pp.tile([128, KT * VTW], BF16, tag="vsb")
            bsb = pp.tile([128, 2], F32, tag="bsb")
            osb = pp.tile([128, KT * 128], F32, tag="osb")
            if with_mask:
                msb = pp.tile([128, KT], F32, tag="msb")

            # x^T streams first, via the cheap Pool sequencer
            for cb in range(CB):
                c0 = cb * 512
                for j in range(8):
                    nc.gpsimd.dma_start(xsb[:, j * S + c0:j * S + c0 + 512],
                                        xt[j * 128:(j + 1) * 128, c0:c0 + 512])
            vsb3 = vsb.rearrange("p (k y) -> p k y", y=VTW)
            nc.gpsimd.memset(vsb3[:, :, 64:65], 1.0)
            nc.gpsimd.memset(vsb3[:, :, 129:130], 1.0)

            nc.vector.memset(qz[64:128, 0:S], 0.0)
            nc.vector.memset(qz[0:64, S:2 * S], 0.0)

            nc.sync.dma_start(bsb[:], bcol[:])
            if with_mask:
                nc.sync.dma_start(msb[:], maskt[:])
            wt = [[None] * 8 for _ in range(3)]
            for proj in range(3):
                for j in range(8):
                    wt[proj][j] = nc_w = pp.tile([128, 128], BF16,
                                                 tag=f"w{proj}_{j}",
                                                 name=f"w{proj}_{j}")
                    nc.sync.dma_start(nc_w[:], w[proj, j * 128:(j + 1) * 128, :])

            # ---- phase A: projections over streamed x blocks ----
            with tc.tile_pool(name="ppsum", bufs=2, space="PSUM") as ppsum:
                for cb in range(CB):
                    c0 = cb * 512
                    pq = ppsum.tile([128, 512], F32, tag="pq", name=f"pq{cb}")
                    for j in range(8):
                        nc.tensor.matmul(pq[:], wt[0][j][:],
                                         xsb[:, j * S + c0:j * S + c0 + 512],
                                         start=(j == 0), stop=(j == 7))
                    nc.vector.tensor_scalar_add(qz[0:64, c0:c0 + 512],
                                                pq[0:64, :], bsb[0:64, 0:1])
                    nc.vector.tensor_scalar_add(qz[64:128, S + c0:S + c0 + 512],
                                                pq[64:128, :], bsb[64:128, 0:1])
                    pk = ppsum.tile([128, 512], F32, tag="pk", name=f"pk{cb}")
                    for j in range(8):
                        nc.tensor.matmul(pk[:], wt[1][j][:],
                                         xsb[:, j * S + c0:j * S + c0 + 512],
                                         start=(j == 0), stop=(j == 7))
                    nc.scalar.activation(ksb[:, c0:c0 + 512], pk[:],
                                         AF.Identity, bias=bsb[:, 1:2])
                    pv = ppsum.tile([128, 512], F32, tag="pv", name=f"pv{cb}")
                    for kk in range(4):
                        for j in range(8):
                            nc.tensor.matmul(
                                pv[:, kk * 128:(kk + 1) * 128],
                                xsb[:, j * S + c0 + kk * 128:
                                    j * S + c0 + (kk + 1) * 128],
                                wt[2][j][:],
                                start=(j == 0), stop=(j == 7))
                    pv3 = pv.rearrange("p (k y) -> p k y", y=128)
                    vd = vsb3[:, cb * 4:(cb + 1) * 4, :]
                    nc.scalar.copy(vd[:, :, 0:64], pv3[:, :, 0:64])
                    nc.scalar.copy(vd[:, :, 65:129], pv3[:, :, 64:128])

            # ---- phase C: attention with per-block DVE epilogue ----
            with (
                tc.tile_pool(name="spoolE", bufs=1, space="PSUM") as spoolE,
                tc.tile_pool(name="spoolO", bufs=1, space="PSUM") as spoolO,
                tc.tile_pool(name="cpsum", bufs=2, space="PSUM") as cpsum,
                tc.tile_pool(name="ppool", bufs=10) as ppool,
                tc.tile_pool(name="rpool", bufs=8) as rpool,
            ):
                def score_block(sp, e, g, q0):
                    for j in range(3):
                        kt = g * 3 + j
                        nc.tensor.matmul(
                            sp[:, j * 512:j * 512 + QW],
                            ksb[:, kt * 128:(kt + 1) * 128],
                            qz[:, e * S + q0:e * S + q0 + QW],
                            start=True, stop=True)

                def exp_block(pt, sp, g):
                    src3 = sp.rearrange("p (g x) -> p g x", x=512)[:, :, 0:QW]
                    dst3 = pt.rearrange("p (g x) -> p g x", x=QW)
                    if with_mask:
                        for j in range(3):
                            kt = g * 3 + j
                            nc.scalar.activation(
                                dst3[:, j, :], src3[:, j, :], AF.Exp,
                                bias=msb[:, kt:kt + 1], scale=0.125)
                    else:
                        nc.scalar.activation(dst3, src3, AF.Exp, scale=0.125)

                def pv_block(cx, pt, h, g):
                    # stationary = exp'd scores [128 keys, 128 q], moving =
                    # v window [128 keys, 65]; out accumulates [q, 65] in
                    # region (h, t3) of the block's single ctx bank.
                    for j in range(3):
                        kt = g * 3 + j
                        for t3 in range(3):
                            r = h * 3 + t3
                            nc.tensor.matmul(
                                cx[:, r * 65:(r + 1) * 65],
                                pt[:, j * QW + t3 * 128:
                                   j * QW + (t3 + 1) * 128],
                                vsb[:, kt * VTW + 65 * h:
                                    kt * VTW + 65 * h + 65],
                                start=(g == 0 and j == 0 and h == 0
                                       and t3 == 0),
                                stop=(g == NG - 1 and j == 2 and h == 1
                                      and t3 == 2),
                                skip_group_check=True)

                def epilogue(qb, cx):
                    for t3 in range(3):
                        t = qb * 3 + t3
                        for h in range(2):
                            r = h * 3 + t3
                            rec = rpool.tile([128, 1], F32, tag="rec",
                                             name=f"rec{qb}_{r}")
                            nc.vector.reciprocal(
                                rec[:], cx[:, r * 65 + 64:r * 65 + 65])
                            nc.vector.tensor_scalar_mul(
                                osb[:, t * 128 + h * 64:t * 128 + h * 64 + 64],
                                cx[:, r * 65:r * 65 + 64], rec[:])
                        nc.sync.dma_start(
                            out[t * 128:(t + 1) * 128, :],
                            osb[:, t * 128:(t + 1) * 128])

                for qb in range(QB):
                    q0 = qb * QW
                    cx = cpsum.tile([128, 512], F32, tag="ctx", name=f"cx{qb}")
                    for g in range(NG):
                        spE = spoolE.tile([128, 1536], F32, tag="spE",
                                          name=f"spE{qb}_{g}")
                        score_block(spE, 0, g, q0)
                        ptE = ppool.tile([128, 3 * QW], BF16, tag="pt",
                                         name=f"ptE{qb}_{g}")
                        exp_block(ptE, spE, g)
                        spO = spoolO.tile([128, 1536], F32, tag="spO",
                                          name=f"spO{qb}_{g}")
                        score_block(spO, 1, g, q0)
                        ptO = ppool.tile([128, 3 * QW], BF16, tag="pt",
                                         name=f"ptO{qb}_{g}")
                        exp_block(ptO, spO, g)
                        pv_block(cx, ptE, 0, g)
                        pv_block(cx, ptO, 1, g)
                    epilogue(qb, cx)

    nc.compile()
    return nc


def _get_program(with_mask: bool):
    key = ("prog", with_mask)
    if key not in _cache:
        _cache[key] = _build(with_mask)
    return _cache[key]


def kernel(hidden_states, attention_mask, Wq, bq, Wk, bk, Wv, bv):
    x = np.asarray(hidden_states, np.float32).reshape(S, HID)
    mask = np.asarray(attention_mask, np.float32).reshape(-1)
    if mask.size == 1:
        mask = np.full(S, float(mask[0]), np.float32)
    with_mask = bool(np.any(mask))

    # transposed weights [3, 1024, 1024]; biases ride separately
    w_all = np.stack([np.asarray(Wq, np.float32).T,
                      np.asarray(Wk, np.float32).T,
                      np.asarray(Wv, np.float32).T]).astype(ml_dtypes.bfloat16)
    bq = np.asarray(bq, np.float32)
    bk = np.asarray(bk, np.float32)
    bv = np.asarray(bv, np.float32)

    xtc = np.ascontiguousarray(x.T).astype(ml_dtypes.bfloat16)
    if with_mask:
        maskt = np.ascontiguousarray(
            mask.reshape(KT, 128).T.astype(np.float32))

    nc = _get_program(with_mask)
    in_maps = []
    for c in range(N_CORES):
        sl = slice(c * 128, (c + 1) * 128)
        m = {
            "xt": xtc,
            "w": np.ascontiguousarray(w_all[:, :, sl]),
            "bcol": np.ascontiguousarray(
                np.stack([bq[sl], bk[sl]], axis=1)),
        }
        if with_mask:
            m["maskt"] = maskt
        in_maps.append(m)

    _cache["last_in_maps"] = in_maps
    res = bass_utils.run_bass_kernel_spmd(nc, in_maps, core_ids=list(range(N_CORES)))
    out = np.concatenate([res.results[c]["out"] for c in range(N_CORES)], axis=1)
    out = out + bv[None, :]
    return out.reshape(B, S, HID).astype(np.float32)
